# revision 2
# baseline (speedup 1.0000x reference)
"""Trainium2 Bass kernel for nn_DecoderLayer (gnn_message_passing).

Reference computation:
    gathered = prev_outputs[parent_indices]            # (K, P, n, n)
    combined = einsum('kp,kpnm->knm', w, gathered) + b
    out      = tanh(combined)[:, None]                 # (K, 1, n, n)
    n_active = prev_is_active[parent_indices].sum(-1)
    out_active = n_active >= 12
    out      = where(out_active, out, 0)

Strategy: the gather+weighted-combine is a sparse (16 nnz/row) matmul
S @ X with S (K, M) built host-side from (parent_indices, w) and
X = prev_outputs.reshape(M, n*n).  The activity gate is folded into S
and b (zeroed rows -> tanh(0) = 0).  K is sharded over 8 cores
(512 nodes each); X (32 MB fp32) is replicated per core.  On device:
dense matmul in fp32r (full PE speed at free-dim 512, ~1e-4 rel err)
accumulating in PSUM over 16 k-tiles, then ScalarE applies
tanh(x + b) straight out of PSUM.
"""
import sys
import numpy as np

sys.path.insert(0, '/opt/trn_rl_repo')

M, K, N, P = 2048, 4096, 64, 16
ACTIVE_THRESHOLD = 12
NCORES = 8
KC = K // NCORES            # 512 nodes per core
E = N * N                   # 4096 matrix elements
QCOLS = 1024                # element columns resident per quarter
NQ = E // QCOLS             # 4 quarters
MT = M // 128               # 16 contraction tiles
NT = KC // 128              # 4 node tiles per core
ECH = QCOLS // 512          # 2 psum chunks per quarter

_cached_nc = None


def _build_program():
    from concourse import bacc, mybir, tile

    nc = bacc.Bacc(None)
    st = nc.dram_tensor("st", [M, KC], mybir.dt.float32r, kind="ExternalInput")
    x = nc.dram_tensor("x", [M, E], mybir.dt.float32r, kind="ExternalInput")
    bia = nc.dram_tensor("bias", [128, NT], mybir.dt.float32, kind="ExternalInput")
    out = nc.dram_tensor("out", [KC, E], mybir.dt.float32, kind="ExternalOutput")

    with tile.TileContext(nc) as tc:
        with tc.tile_pool(name="stp", bufs=1) as stp, \
             tc.tile_pool(name="xp", bufs=2) as xp, \
             tc.tile_pool(name="bp", bufs=1) as bp, \
             tc.tile_pool(name="op", bufs=4) as op, \
             tc.tile_pool(name="ps", bufs=1, space="PSUM") as ps:
            b_sb = bp.tile([128, NT], mybir.dt.float32)
            nc.sync.dma_start(b_sb[:], bia[:])

            st_sb = [None] * MT
            for q in range(NQ):
                xt = []
                for t in range(MT):
                    # Interleave ST and X loads so the t=0 matmuls can
                    # start after the first pair lands, not after 12 MB.
                    if q == 0:
                        stt = stp.tile([128, KC], mybir.dt.float32r,
                                       tag=f"st{t}", name=f"st{t}")
                        nc.sync.dma_start(stt[:], st[t * 128:(t + 1) * 128, :])
                        st_sb[t] = stt
                    xtile = xp.tile([128, QCOLS], mybir.dt.float32r,
                                    tag=f"x{t}", name=f"x{q}_{t}")
                    nc.sync.dma_start(
                        xtile[:], x[t * 128:(t + 1) * 128, q * QCOLS:(q + 1) * QCOLS])
                    xt.append(xtile)
                # 8 concurrent accumulation groups, one PSUM bank each;
                # contraction loop outermost keeps PE dense from the start.
                accs = [[ps.tile([128, 512], mybir.dt.float32,
                                 tag=f"acc{n}_{e}", name=f"acc{q}_{n}_{e}")
                         for e in range(ECH)] for n in range(NT)]
                for t in range(MT):
                    for n in range(NT):
                        for e in range(ECH):
                            nc.tensor.matmul(
                                accs[n][e][:],
                                st_sb[t][:, n * 128:(n + 1) * 128],
                                xt[t][:, e * 512:(e + 1) * 512],
                                start=(t == 0), stop=(t == MT - 1))
                for n in range(NT):
                    for e in range(ECH):
                        o_sb = op.tile([128, 512], mybir.dt.float32,
                                       tag="osb", name=f"o{q}_{n}_{e}")
                        nc.scalar.activation(
                            o_sb[:], accs[n][e][:],
                            mybir.ActivationFunctionType.Tanh,
                            bias=b_sb[:, n:n + 1])
                        col = q * QCOLS + e * 512
                        nc.sync.dma_start(
                            out[n * 128:(n + 1) * 128, col:col + 512], o_sb[:])
    nc.finalize()
    return nc


def _prepare_host(prev_outputs, prev_is_active, parent_indices, w, b):
    po = np.asarray(prev_outputs, dtype=np.float32)
    act = np.asarray(prev_is_active).astype(bool)
    idx = np.asarray(parent_indices)
    wf = np.asarray(w, dtype=np.float32)
    bf = np.asarray(b, dtype=np.float32)

    n_active = act[idx].astype(np.int32).sum(axis=-1)
    out_active = n_active >= ACTIVE_THRESHOLD

    # Dense scatter of weights; duplicate parents accumulate like the einsum.
    S = np.zeros((K, M), dtype=np.float32)
    rows = np.repeat(np.arange(K), P)
    np.add.at(S, (rows, idx.reshape(-1)), wf.reshape(-1))
    S[~out_active] = 0.0
    b_masked = np.where(out_active, bf, 0.0).astype(np.float32)

    ST = np.ascontiguousarray(S.T)              # (M, K)
    X = np.ascontiguousarray(po.reshape(M, E))  # (M, E)
    return ST, X, b_masked, out_active


def kernel(prev_outputs, prev_is_active, parent_indices, w, b,
           _debug_results=None):
    from concourse.bass_utils import run_bass_kernel_spmd

    ST, X, b_masked, out_active = _prepare_host(
        prev_outputs, prev_is_active, parent_indices, w, b)

    global _cached_nc
    if _cached_nc is None:
        _cached_nc = _build_program()
    nc = _cached_nc

    in_maps = []
    for c in range(NCORES):
        sl = slice(c * KC, (c + 1) * KC)
        in_maps.append({
            "st": np.ascontiguousarray(ST[:, sl]),
            "x": X,
            "bias": np.ascontiguousarray(b_masked[sl].reshape(NT, 128).T),
        })

    kw = dict(_debug_results) if _debug_results else {}
    res = run_bass_kernel_spmd(nc, in_maps, list(range(NCORES)), **kw)
    outs = [res.results[c]["out"] for c in range(NCORES)]
    out_full = np.concatenate(outs, axis=0).reshape(K, 1, N, N)
    if _debug_results:
        return (out_full.astype(np.float32), out_active), res
    return out_full.astype(np.float32), out_active


# revision 3
# speedup vs baseline: 1.0476x; 1.0476x over previous
"""Trainium2 Bass kernel for nn_DecoderLayer (gnn_message_passing).

Reference computation:
    gathered = prev_outputs[parent_indices]            # (K, P, n, n)
    combined = einsum('kp,kpnm->knm', w, gathered) + b
    out      = tanh(combined)[:, None]                 # (K, 1, n, n)
    n_active = prev_is_active[parent_indices].sum(-1)
    out_active = n_active >= 12
    out      = where(out_active, out, 0)

Strategy: the gather+weighted-combine is a sparse (16 nnz/row) matmul
S @ X with S (K, M) built host-side from (parent_indices, w) and
X = prev_outputs.reshape(M, n*n).  The activity gate is folded into S
and b (zeroed rows -> tanh(0) = 0).  K is sharded over 8 cores
(512 nodes each); X (32 MB fp32) is replicated per core.  On device:
dense matmul in fp32r (full PE speed at free-dim 512, ~1e-4 rel err)
accumulating in PSUM over 16 k-tiles, then ScalarE applies
tanh(x + b) straight out of PSUM.
"""
import sys
import numpy as np

sys.path.insert(0, '/opt/trn_rl_repo')

M, K, N, P = 2048, 4096, 64, 16
ACTIVE_THRESHOLD = 12
NCORES = 8
KC = K // NCORES            # 512 nodes per core
E = N * N                   # 4096 matrix elements
QCOLS = 1024                # element columns resident per quarter
NQ = E // QCOLS             # 4 quarters
MT = M // 128               # 16 contraction tiles
NT = KC // 128              # 4 node tiles per core
ECH = QCOLS // 512          # 2 psum chunks per quarter

_cached_nc = None


def _build_program():
    from concourse import bacc, mybir, tile

    nc = bacc.Bacc(None)
    st = nc.dram_tensor("st", [M, KC], mybir.dt.float32r, kind="ExternalInput")
    x = nc.dram_tensor("x", [M, E], mybir.dt.float32r, kind="ExternalInput")
    bia = nc.dram_tensor("bias", [128, NT], mybir.dt.float32, kind="ExternalInput")
    out = nc.dram_tensor("out", [KC, E], mybir.dt.float32, kind="ExternalOutput")

    CH = 4                  # contraction tiles per DMA chunk
    NC_CH = MT // CH        # 4 chunks
    TSPLIT = MT - 4         # phase A: t<TSPLIT all groups; B: per group

    with tile.TileContext(nc) as tc:
        with tc.tile_pool(name="stp", bufs=1) as stp, \
             tc.tile_pool(name="xp", bufs=2) as xp, \
             tc.tile_pool(name="bp", bufs=1) as bp, \
             tc.tile_pool(name="op", bufs=4) as op, \
             tc.tile_pool(name="ps", bufs=1, space="PSUM") as ps:
            b_sb = bp.tile([128, NT], mybir.dt.float32)
            nc.scalar.dma_start(b_sb[:], bia[:])

            # ST chunks: (128, CH*KC); sub-DMA'd tile-by-tile on the
            # scalar queue while X streams on the sync queue.
            st_ch = []
            for c in range(NC_CH):
                sc = stp.tile([128, CH * KC], mybir.dt.float32r,
                              tag=f"st{c}", name=f"st{c}")
                st_ch.append(sc)
            for t in range(MT):
                c, i = t // CH, t % CH
                nc.scalar.dma_start(
                    st_ch[c][:, i * KC:(i + 1) * KC],
                    st[t * 128:(t + 1) * 128, :])

            def st_ap(t, n):
                c, i = t // CH, t % CH
                return st_ch[c][:, i * KC + n * 128: i * KC + (n + 1) * 128]

            for q in range(NQ):
                xc = [xp.tile([128, CH * QCOLS], mybir.dt.float32r,
                              tag=f"x{c}", name=f"x{q}_{c}")
                      for c in range(NC_CH)]
                if q == 0:
                    # fine granularity so the first matmuls start early
                    for t in range(MT):
                        c, i = t // CH, t % CH
                        nc.sync.dma_start(
                            xc[c][:, i * QCOLS:(i + 1) * QCOLS],
                            x[t * 128:(t + 1) * 128, q * QCOLS:(q + 1) * QCOLS])
                else:
                    for c in range(NC_CH):
                        src = x[c * CH * 128:(c + 1) * CH * 128,
                                q * QCOLS:(q + 1) * QCOLS]
                        nc.sync.dma_start(
                            xc[c][:].rearrange("p (t c) -> p t c", t=CH),
                            src.rearrange("(t p) c -> p t c", p=128))

                def x_ap(t, e):
                    c, i = t // CH, t % CH
                    off = i * QCOLS + e * 512
                    return xc[c][:, off:off + 512]

                accs = [[ps.tile([128, 512], mybir.dt.float32,
                                 tag=f"acc{n}_{e}", name=f"acc{q}_{n}_{e}")
                         for e in range(ECH)] for n in range(NT)]
                # Phase A: contraction-outer across all 8 PSUM banks.
                for t in range(TSPLIT):
                    for n in range(NT):
                        for e in range(ECH):
                            nc.tensor.matmul(
                                accs[n][e][:], st_ap(t, n), x_ap(t, e),
                                start=(t == 0), stop=False)
                # Phase B: finish groups one at a time so ACT + store
                # overlap the remaining matmuls instead of bunching.
                for n in range(NT):
                    for e in range(ECH):
                        for t in range(TSPLIT, MT):
                            nc.tensor.matmul(
                                accs[n][e][:], st_ap(t, n), x_ap(t, e),
                                start=False, stop=(t == MT - 1))
                        o_sb = op.tile([128, 512], mybir.dt.float32,
                                       tag="osb", name=f"o{q}_{n}_{e}")
                        nc.scalar.activation(
                            o_sb[:], accs[n][e][:],
                            mybir.ActivationFunctionType.Tanh,
                            bias=b_sb[:, n:n + 1])
                        col = q * QCOLS + e * 512
                        nc.scalar.dma_start(
                            out[n * 128:(n + 1) * 128, col:col + 512], o_sb[:])
    nc.finalize()
    return nc


def _prepare_host(prev_outputs, prev_is_active, parent_indices, w, b):
    po = np.asarray(prev_outputs, dtype=np.float32)
    act = np.asarray(prev_is_active).astype(bool)
    idx = np.asarray(parent_indices)
    wf = np.asarray(w, dtype=np.float32)
    bf = np.asarray(b, dtype=np.float32)

    n_active = act[idx].astype(np.int32).sum(axis=-1)
    out_active = n_active >= ACTIVE_THRESHOLD

    # Dense scatter of weights; duplicate parents accumulate like the einsum.
    S = np.zeros((K, M), dtype=np.float32)
    rows = np.repeat(np.arange(K), P)
    np.add.at(S, (rows, idx.reshape(-1)), wf.reshape(-1))
    S[~out_active] = 0.0
    b_masked = np.where(out_active, bf, 0.0).astype(np.float32)

    ST = np.ascontiguousarray(S.T)              # (M, K)
    X = np.ascontiguousarray(po.reshape(M, E))  # (M, E)
    return ST, X, b_masked, out_active


def kernel(prev_outputs, prev_is_active, parent_indices, w, b,
           _debug_results=None):
    from concourse.bass_utils import run_bass_kernel_spmd

    ST, X, b_masked, out_active = _prepare_host(
        prev_outputs, prev_is_active, parent_indices, w, b)

    global _cached_nc
    if _cached_nc is None:
        _cached_nc = _build_program()
    nc = _cached_nc

    in_maps = []
    for c in range(NCORES):
        sl = slice(c * KC, (c + 1) * KC)
        in_maps.append({
            "st": np.ascontiguousarray(ST[:, sl]),
            "x": X,
            "bias": np.ascontiguousarray(b_masked[sl].reshape(NT, 128).T),
        })

    kw = dict(_debug_results) if _debug_results else {}
    res = run_bass_kernel_spmd(nc, in_maps, list(range(NCORES)), **kw)
    outs = [res.results[c]["out"] for c in range(NCORES)]
    out_full = np.concatenate(outs, axis=0).reshape(K, 1, N, N)
    if _debug_results:
        return (out_full.astype(np.float32), out_active), res
    return out_full.astype(np.float32), out_active


# revision 4
# speedup vs baseline: 1.1425x; 1.0905x over previous
"""Trainium2 Bass kernel for nn_DecoderLayer (gnn_message_passing).

Reference computation:
    gathered = prev_outputs[parent_indices]            # (K, P, n, n)
    combined = einsum('kp,kpnm->knm', w, gathered) + b
    out      = tanh(combined)[:, None]                 # (K, 1, n, n)
    n_active = prev_is_active[parent_indices].sum(-1)
    out_active = n_active >= 12
    out      = where(out_active, out, 0)

Strategy: the gather+weighted-combine is a sparse (16 nnz/row) matmul
S @ X with S (K, M) built host-side from (parent_indices, w) and
X = prev_outputs.reshape(M, n*n).  The activity gate is folded into S
and b (zeroed rows -> tanh(0) = 0).  K is sharded over 8 cores
(512 nodes each); X (32 MB fp32) is replicated per core.  On device:
dense matmul in fp32r (full PE speed at free-dim 512, ~1e-4 rel err)
accumulating in PSUM over 16 k-tiles, then ScalarE applies
tanh(x + b) straight out of PSUM.
"""
import sys
import numpy as np

sys.path.insert(0, '/opt/trn_rl_repo')

M, K, N, P = 2048, 4096, 64, 16
ACTIVE_THRESHOLD = 12
NCORES = 8
KC = K // NCORES            # 512 nodes per core
E = N * N                   # 4096 matrix elements
QCOLS = 1024                # element columns resident per quarter
NQ = E // QCOLS             # 4 quarters
MT = M // 128               # 16 contraction tiles
NT = KC // 128              # 4 node tiles per core
ECH = QCOLS // 512          # 2 psum chunks per quarter

_cached_nc = None


def _build_program():
    from concourse import bacc, mybir, tile

    nc = bacc.Bacc(None)
    st = nc.dram_tensor("st", [M, KC], mybir.dt.float32r, kind="ExternalInput")
    x = nc.dram_tensor("x", [M, E], mybir.dt.float32r, kind="ExternalInput")
    bia = nc.dram_tensor("bias", [128, NT], mybir.dt.float32, kind="ExternalInput")
    out = nc.dram_tensor("out", [KC, E], mybir.dt.float32, kind="ExternalOutput")

    CH = 4                  # contraction tiles per DMA chunk
    NC_CH = MT // CH        # 4 chunks
    TSPLIT = MT - 4         # phase A: t<TSPLIT all groups; B: per group

    with tile.TileContext(nc) as tc:
        with tc.tile_pool(name="stp", bufs=1) as stp, \
             tc.tile_pool(name="xp", bufs=2) as xp, \
             tc.tile_pool(name="bp", bufs=1) as bp, \
             tc.tile_pool(name="op", bufs=4) as op, \
             tc.tile_pool(name="ps", bufs=1, space="PSUM") as ps:
            b_sb = bp.tile([128, NT], mybir.dt.float32)
            nc.scalar.dma_start(b_sb[:], bia[:])

            # Per-tile ST/X tensors: Tile tracks readiness per tile, so
            # fine granularity lets the t=0 matmuls start after ~0.75 MB.
            st_sb = []
            for t in range(MT):
                stt = stp.tile([128, KC], mybir.dt.float32r,
                               tag=f"st{t}", name=f"st{t}")
                nc.scalar.dma_start(stt[:], st[t * 128:(t + 1) * 128, :])
                st_sb.append(stt)

            def st_ap(t, n):
                return st_sb[t][:, n * 128:(n + 1) * 128]

            for q in range(NQ):
                xt = []
                for t in range(MT):
                    xtile = xp.tile([128, QCOLS], mybir.dt.float32r,
                                    tag=f"x{t}", name=f"x{q}_{t}")
                    nc.sync.dma_start(
                        xtile[:],
                        x[t * 128:(t + 1) * 128, q * QCOLS:(q + 1) * QCOLS])
                    xt.append(xtile)

                def x_ap(t, e):
                    return xt[t][:, e * 512:(e + 1) * 512]

                accs = [[ps.tile([128, 512], mybir.dt.float32,
                                 tag=f"acc{n}_{e}", name=f"acc{q}_{n}_{e}")
                         for e in range(ECH)] for n in range(NT)]
                # Phase A: contraction-outer across all 8 PSUM banks.
                for t in range(TSPLIT):
                    for n in range(NT):
                        for e in range(ECH):
                            nc.tensor.matmul(
                                accs[n][e][:], st_ap(t, n), x_ap(t, e),
                                start=(t == 0), stop=False)
                # Phase B: finish groups one at a time so ACT + store
                # overlap the remaining matmuls instead of bunching.
                for n in range(NT):
                    for e in range(ECH):
                        for t in range(TSPLIT, MT):
                            nc.tensor.matmul(
                                accs[n][e][:], st_ap(t, n), x_ap(t, e),
                                start=False, stop=(t == MT - 1))
                        o_sb = op.tile([128, 512], mybir.dt.float32,
                                       tag="osb", name=f"o{q}_{n}_{e}")
                        nc.scalar.activation(
                            o_sb[:], accs[n][e][:],
                            mybir.ActivationFunctionType.Tanh,
                            bias=b_sb[:, n:n + 1])
                        col = q * QCOLS + e * 512
                        nc.scalar.dma_start(
                            out[n * 128:(n + 1) * 128, col:col + 512], o_sb[:])
    nc.finalize()
    return nc


def _prepare_host(prev_outputs, prev_is_active, parent_indices, w, b):
    po = np.asarray(prev_outputs, dtype=np.float32)
    act = np.asarray(prev_is_active).astype(bool)
    idx = np.asarray(parent_indices)
    wf = np.asarray(w, dtype=np.float32)
    bf = np.asarray(b, dtype=np.float32)

    n_active = act[idx].astype(np.int32).sum(axis=-1)
    out_active = n_active >= ACTIVE_THRESHOLD

    # Dense scatter of weights; duplicate parents accumulate like the einsum.
    S = np.zeros((K, M), dtype=np.float32)
    rows = np.repeat(np.arange(K), P)
    np.add.at(S, (rows, idx.reshape(-1)), wf.reshape(-1))
    S[~out_active] = 0.0
    b_masked = np.where(out_active, bf, 0.0).astype(np.float32)

    ST = np.ascontiguousarray(S.T)              # (M, K)
    X = np.ascontiguousarray(po.reshape(M, E))  # (M, E)
    return ST, X, b_masked, out_active


def kernel(prev_outputs, prev_is_active, parent_indices, w, b,
           _debug_results=None):
    from concourse.bass_utils import run_bass_kernel_spmd

    ST, X, b_masked, out_active = _prepare_host(
        prev_outputs, prev_is_active, parent_indices, w, b)

    global _cached_nc
    if _cached_nc is None:
        _cached_nc = _build_program()
    nc = _cached_nc

    in_maps = []
    for c in range(NCORES):
        sl = slice(c * KC, (c + 1) * KC)
        in_maps.append({
            "st": np.ascontiguousarray(ST[:, sl]),
            "x": X,
            "bias": np.ascontiguousarray(b_masked[sl].reshape(NT, 128).T),
        })

    kw = dict(_debug_results) if _debug_results else {}
    res = run_bass_kernel_spmd(nc, in_maps, list(range(NCORES)), **kw)
    outs = [res.results[c]["out"] for c in range(NCORES)]
    out_full = np.concatenate(outs, axis=0).reshape(K, 1, N, N)
    if _debug_results:
        return (out_full.astype(np.float32), out_active), res
    return out_full.astype(np.float32), out_active
